# revision 51
# baseline (speedup 1.0000x reference)
"""Trainium2 Bass kernel v34 (from v24 baseline 360us -> 274us).

Data-parallel over batch across 8 cores (16 seq/core), batch pairs with
deep cross-pair software pipelining:

  attention(N) is FUSED with ffn1(N-1) chunks so the PE chews ffn matmuls
  while exp/affine-select/rcp latencies drain (no >3.4us PE idle -> the
  HAM clock gate stays at 2.4GHz); ffn2(N-1) covers the ln2 stats chain;
  all PE-transposes are deferred until the next pair's matmuls are queued
  (no FIFO head-of-line stalls).

  LayerNorm: DVE bn_stats + cubic-seed Newton rsqrt; the APPLY runs on the
  otherwise-idle GpSimd engine; PE transpose-mode builds xnT/hnT bf16.
  Scores: [128, 3x128] contiguous psum (keys0 x q0:256, keys1 x q128:256);
  adjacent heads alternate PE row groups (po 0/64) so their K=64 matmuls
  run CONCURRENTLY in the array; ONE exp per (bi, head); causal mask via
  gpsimd affine_select on the two diagonal blocks.  av: bf16 with a
  64-wide ones block giving softmax denominators in the same matmul;
  normalize = batched rcp+mult over [64, 2T] per head.

  All GEMMs bf16 except ffn2's K-planes 6-11 which run fp8e4 DoubleRow
  (2 K-planes/pass): h1 planes 6-11 evac x32 into fp8, w2 planes 6-11
  x32 fp8, w2 planes 0-5 x1024 bf16 (exact shift) so both halves land
  x1024 in PSUM and one evac rescale covers them.  Quantizing only half
  of ffn2's K keeps rel-err at 1.53e-2 (< 2e-2 gate).
"""

import os
import sys

for _p in ("/opt/trn_rl_repo", "/root/.axon_site/_ro/trn_rl_repo"):
    if os.path.isdir(_p) and _p not in sys.path:
        sys.path.append(_p)

import numpy as np


def _q8(a):
    """TRN fp8e4 (max +-240) via OCP e4m3fn (bit-compatible below 256)."""
    import ml_dtypes
    return np.clip(np.asarray(a, np.float32), -240.0, 240.0).astype(
        ml_dtypes.float8_e4m3fn)


import concourse.bass as bass  # noqa: F401
import concourse.tile as tile
from concourse import bacc, mybir
from concourse.bass_utils import run_bass_kernel_spmd

f32 = mybir.dt.float32
bf16 = mybir.dt.bfloat16
f8 = mybir.dt.float8e4
AF = mybir.ActivationFunctionType
ALU = mybir.AluOpType

N_CORES = 8
B, T, C = 128, 256, 384
H, D = 6, 64
F = 4 * C  # 1536
BPC = B // N_CORES  # 16 sequences per core
LN_EPS = 1e-5
ESC = float(C) ** -0.5

RELU_DVE = 2       # how many of the 12 ffn1 relu evacs run on DVE
WARM_MM = 36       # real-matmul warmup count

TRACE = False
_CACHE = {}


def _build(bias_flags):
    has_qb, has_kb, has_vb, has_bp, has_b2, has_b1 = bias_flags
    relu_dve = 0 if has_b1 else RELU_DVE

    nc = bacc.Bacc("TRN2", target_bir_lowering=False, debug=False)

    x_d = nc.dram_tensor("x", (BPC, T, C), f32, kind="ExternalInput").ap()
    wq_d = nc.dram_tensor("wq", (128, 3, C), bf16, kind="ExternalInput").ap()
    wk_d = nc.dram_tensor("wk", (128, 3, C), bf16, kind="ExternalInput").ap()
    wv_d = nc.dram_tensor("wv", (128, 3, C), bf16, kind="ExternalInput").ap()
    wp_d = nc.dram_tensor("wp", (128, 3, C), bf16, kind="ExternalInput").ap()
    w1_d = nc.dram_tensor("w1", (128, 3, F), bf16, kind="ExternalInput").ap()
    # ffn2 split-K: planes 0-5 bf16 scaled x1024, planes 6-11 fp8e4 x32
    # (x32 h1 side too -> both halves land x1024 in PSUM; evac divides)
    w2a_d = nc.dram_tensor("w2a", (128, 6, C), bf16, kind="ExternalInput").ap()
    w2b_d = nc.dram_tensor("w2b", (128, 6, C), f8, kind="ExternalInput").ap()
    b1_d = nc.dram_tensor("b1", (F,), f32, kind="ExternalInput").ap()
    bias_d = nc.dram_tensor("biases", (5, C), bf16, kind="ExternalInput").ap()
    out_d = nc.dram_tensor("out", (BPC, T, C), f32, kind="ExternalOutput").ap()

    identb_d = nc.inline_tensor(np.eye(128, dtype=np.float32), name="identc").ap()

    with tile.TileContext(nc) as tc:
        with tc.tile_pool(name="wpool", bufs=1) as wpool, \
             tc.tile_pool(name="pool", bufs=2) as pool, \
             tc.tile_pool(name="ppool", bufs=8, space="PSUM") as ppool:

            identf = wpool.tile([128, 128], f32)
            nc.sync.dma_start(identf[:], identb_d[:])
            ident = wpool.tile([128, 128], bf16)
            nc.vector.tensor_copy(ident[:], identf[:])
            # warm the PE p-state with REAL matmuls (transpose-mode does not
            # engage the HAM activity monitor) while weight/x DMAs stream in
            for _w in range(WARM_MM):
                wps = ppool.tile([128, 128], f32, tag="ps", name="warm")
                nc.tensor.matmul(wps[:], ident[:], ident[:], start=True,
                                 stop=True)

            wq = wpool.tile([128, 3, C], bf16)
            wk = wpool.tile([128, 3, C], bf16)
            wv = wpool.tile([128, 3, C], bf16)
            wp = wpool.tile([128, 3, C], bf16)
            w1 = wpool.tile([128, 3, F], bf16)
            w2a = wpool.tile([128, 6, C], bf16)
            w2b = wpool.tile([128, 6, C], f8)
            b1v = wpool.tile([128, 12], f32)
            ones8 = wpool.tile([128, 512], bf16)
            nc.gpsimd.memset(ones8[:], 1.0)
            biases = wpool.tile([128, 5, C], bf16)
            qb, kb, vb, bpj, b2b = (biases[0:1, i, :] for i in range(5))

            def load_weights():
                # issued AFTER the first x DMAs so the pipeline's head isn't
                # starved behind ~2.4MB of weights
                nc.sync.dma_start(wq[:], wq_d[:])
                nc.sync.dma_start(wk[:], wk_d[:])
                nc.sync.dma_start(wv[:], wv_d[:])
                nc.sync.dma_start(wp[:], wp_d[:])
                nc.sync.dma_start(w1[:], w1_d[:])
                nc.sync.dma_start(w2a[:], w2a_d[:])
                nc.sync.dma_start(w2b[:], w2b_d[:])
                nc.sync.dma_start(b1v[:], b1_d.rearrange("(m p) -> p m", p=128))
                nc.sync.dma_start(biases[0:1], bias_d[None, :, :])

            def rsqrt_newton(y, we, tag):
                """y ~ rsqrt(we): cubic seed + one Newton pass -> 2.8e-4."""
                a = pool.tile(y.shape, f32, tag=tag + "_a", name="rs_a", bufs=2)
                nc.vector.tensor_scalar(a[:], we, -0.0461311, 0.3783969,
                                        ALU.mult, ALU.add)
                nc.vector.tensor_tensor(a[:], a[:], we, ALU.mult)
                nc.vector.scalar_tensor_tensor(a[:], a[:], -1.1281522, we,
                                               ALU.add, ALU.mult)
                nc.vector.tensor_scalar(y, a[:], 1.8094985, None, ALU.add)
                nc.vector.tensor_tensor(a[:], y, y, ALU.mult)
                nc.vector.tensor_tensor(a[:], a[:], we, ALU.mult)
                nc.vector.tensor_scalar(a[:], a[:], -0.5, 1.5, ALU.mult,
                                        ALU.add)
                nc.vector.tensor_tensor(y, y, a[:], ALU.mult)

            def ln_stats_a(srcs, tag):
                """DVE-only part: srcs: per (bi, kt) [128, C] views ->
                rstd4 [128,4] f32, nb4 [128,4] f32 = -mu*rstd."""
                var4 = pool.tile([128, 4, 2], f32, tag=tag + "_v", name="var4")
                for i, src in enumerate(srcs):
                    stats = pool.tile([128, 6], f32, tag=tag + "_s",
                                      name="stats", bufs=6)
                    nc.vector.bn_stats(stats[:], src)
                    nc.vector.bn_aggr(var4[:, i], stats[:])
                we = pool.tile([128, 4], f32, tag=tag + "_w", name="we")
                nc.vector.tensor_scalar(we[:], var4[:, :, 1], LN_EPS, None,
                                        ALU.add)
                rstd4 = pool.tile([128, 4], f32, tag=tag + "_r", name="rstd4",
                                  bufs=3)
                rsqrt_newton(rstd4[:], we[:], tag)
                nb4 = pool.tile([128, 4], f32, tag=tag + "_b", name="nb4",
                                bufs=3)
                nc.vector.scalar_tensor_tensor(nb4[:], var4[:, :, 0], -1.0,
                                               rstd4[:], ALU.mult, ALU.mult)
                return rstd4, nb4

            def lnT(dst, srcs, rstd4, nb4, tag, evac_dve=False):
                """dst [128, 3, 512] bf16: LN apply on GpSimd (idle engine)
                then PE transpose-mode + evac."""
                xn = pool.tile([128, 4, C], bf16, tag=tag + "_n", name="xn",
                               bufs=3)
                for i in range(4):
                    nc.gpsimd.tensor_scalar(xn[:, i], srcs[i],
                                            rstd4[:, i:i + 1],
                                            nb4[:, i:i + 1],
                                            ALU.mult, ALU.add)
                for i in range(4):
                    tp = ppool.tile([128, 3, 128], bf16, tag="ps", name="lnT")
                    for c in range(3):
                        nc.tensor.transpose(
                            tp[:, c], xn[:, i, c * 128:(c + 1) * 128],
                            ident[:])
                    if evac_dve:
                        nc.vector.tensor_copy(
                            dst[:, :, i * 128:(i + 1) * 128], tp[:])
                    else:
                        nc.scalar.copy(dst[:, :, i * 128:(i + 1) * 128], tp[:])

            def ln1_dma(bp):
                pair = (2 * bp, 2 * bp + 1)
                xs = []
                for b in pair:
                    x_t = pool.tile([128, 2, C], f32, tag="x", name="x_t",
                                    bufs=6)
                    for kt in range(2):
                        nc.sync.dma_start(x_t[:, kt],
                                          x_d[b, kt * 128:(kt + 1) * 128, :])
                    xs.append(x_t)
                srcs = [xs[bi][:, kt] for bi in range(2) for kt in range(2)]
                rstd4, nb4 = ln_stats_a(srcs, "ln1")
                return {"pair": pair, "xs": xs, "ln1a": (rstd4, nb4)}

            def tp_qkv(st):
                rstd4, nb4 = st.pop("ln1a")
                xnT = pool.tile([128, 3, 2 * T], bf16, tag="xnT", name="xnT",
                                bufs=3)
                srcs = [st["xs"][bi][:, kt] for bi in range(2)
                        for kt in range(2)]
                lnT(xnT, srcs, rstd4, nb4, "ln1")

                qT = pool.tile([128, 3, 2 * T], bf16, tag="qT", name="qT")
                kT = pool.tile([128, 3, 2 * T], bf16, tag="kT", name="kT")
                for dst, w, hb, hasb in ((qT, wq, qb, has_qb),
                                         (kT, wk, kb, has_kb)):
                    for m in range(3):
                        ps = ppool.tile([128, 2 * T], f32, tag="ps",
                                        name="qk_ps")
                        for c in range(3):
                            nc.tensor.matmul(ps[:], w[:, c, m * 128:(m + 1) * 128],
                                             xnT[:, c, :], start=(c == 0),
                                             stop=(c == 2 and not hasb))
                        if hasb:
                            nc.tensor.matmul(ps[:], hb[:, m * 128:(m + 1) * 128],
                                             ones8[0:1, :], start=False,
                                             stop=True)
                        nc.scalar.copy(dst[:, m, :], ps[:])
                v8s = []
                for bi in range(2):
                    v8 = pool.tile([128, 2, H, 128], bf16, tag="v8", name="v8")
                    if st["pair"][bi] < 4:  # first pass over each buffer
                        nc.gpsimd.memset(v8[:, :, :, 0:64], 1.0)
                    for kt in range(2):
                        tk = 2 * bi + kt
                        ps = ppool.tile([128, C], f32, tag="ps", name="v_ps")
                        for c in range(3):
                            nc.tensor.matmul(
                                ps[:], xnT[:, c, tk * 128:(tk + 1) * 128],
                                wv[:, c, :], start=(c == 0),
                                stop=(c == 2 and not has_vb))
                        if has_vb:
                            nc.tensor.matmul(ps[:], ones8[0:1, 0:128], vb,
                                             start=False, stop=True)
                        nc.scalar.copy(
                            v8[:, kt, :, 64:128],
                            ps[:].rearrange("p (h d) -> p h d", d=D))
                    v8s.append(v8)
                st.update(xnT=xnT, qT=qT, kT=kT, v8s=v8s)
                return st

            def ffn1_chunk(st, lo, hi):
                """ffn1 matmuls for mf in [lo, hi) + relu evac. h1 planes
                0-5 evac bf16; planes 6-11 evac fp8e4 scaled x32."""
                hnT = st["hnT"]
                if "h1a" not in st:
                    st["h1a"] = pool.tile([128, 6, 2 * T], bf16, tag="h1a",
                                          name="h1a", bufs=1)
                    st["h1b"] = pool.tile([128, 6, 2 * T], f8, tag="h1b",
                                          name="h1b", bufs=1)
                for mf in range(lo, hi):
                    ps = ppool.tile([128, 2 * T], f32, tag="ps", name="f1_ps")
                    for c in range(3):
                        nc.tensor.matmul(ps[:], w1[:, c, mf * 128:(mf + 1) * 128],
                                         hnT[:, c, :], start=(c == 0),
                                         stop=(c == 2))
                    dst = st["h1a"][:, mf, :] if mf < 6 else \
                        st["h1b"][:, mf - 6, :]
                    sc8 = 1.0 if mf < 6 else 32.0
                    if mf < 12 - relu_dve:
                        nc.scalar.activation(dst, ps[:], AF.Relu,
                                             bias=b1v[:, mf:mf + 1], scale=sc8)
                    else:  # only used when b1 == 0
                        nc.vector.tensor_scalar(dst, ps[:], sc8, 0.0,
                                                ALU.mult, ALU.max)

            def attention(st, prev):
                """Attention for st, interleaved with ffn1 chunks of prev so
                the PE never idles while exp/select/rcp latencies drain."""
                qT, kT, v8s = st["qT"], st["kT"], st["v8s"]
                attnT = pool.tile([128, 3, 2 * T], bf16, tag="attnT",
                                  name="attnT")
                # adjacent heads use PE row groups 0/64 -> their K=64 score
                # matmuls run concurrently; issue them back-to-back.
                for hp in range(3):
                    weiTs = {}
                    scs = {}
                    for dh in range(2):
                        h = 2 * hp + dh
                        po = dh * 64
                        for bi in range(2):
                            q0 = bi * T
                            sc = ppool.tile([128, 3, 128], f32, tag="ps",
                                            name="sc")
                            nc.tensor.matmul(
                                sc[:, 0:2].rearrange("p a b -> p (a b)"),
                                kT[po:po + 64, hp, q0:q0 + 128],
                                qT[po:po + 64, hp, q0:q0 + T],
                                start=True, stop=False)
                            nc.tensor.matmul(
                                sc[:, 2],
                                kT[po:po + 64, hp, q0 + 128:q0 + 256],
                                qT[po:po + 64, hp, q0 + 128:q0 + 256],
                                start=False, stop=True)
                            scs[(h, bi)] = sc
                    for dh in range(2):
                        h = 2 * hp + dh
                        for bi in range(2):
                            weiT = pool.tile([128, 3, 128], bf16, tag="weiT",
                                             name="weiT", bufs=10)
                            nc.scalar.activation(weiT[:], scs[(h, bi)][:],
                                                 AF.Exp, scale=ESC)
                            for blk in (0, 2):  # causal mask on diag blocks
                                nc.gpsimd.affine_select(
                                    out=weiT[:, blk], in_=weiT[:, blk],
                                    compare_op=ALU.is_ge, fill=0.0, base=0,
                                    pattern=[[1, 128]], channel_multiplier=-1)
                            weiTs[(h, bi)] = weiT
                    # PE filler while exp/select latency drains
                    if prev is not None:
                        ffn1_chunk(prev, 4 * hp, 4 * hp + 4)
                    else:  # first pair: keep PE busy/HAM warm with dummies
                        for _ in range(10):
                            wps = ppool.tile([128, 128], f32, tag="ps",
                                             name="warm")
                            nc.tensor.matmul(wps[:], ident[:], ident[:],
                                             start=True, stop=True)
                    for dh in range(2):
                        h = 2 * hp + dh
                        po = dh * 64
                        av = ppool.tile([128, 2, T], f32, tag="ps", name="av")
                        for bi in range(2):
                            nc.tensor.matmul(
                                av[:, bi],
                                v8s[bi][:, 0, h, :],
                                weiTs[(h, bi)][:, 0:2].rearrange("p a b -> p (a b)"),
                                start=(bi == 0), stop=False)
                            nc.tensor.matmul(
                                av[:, bi, 128:256],
                                v8s[bi][:, 1, h, :], weiTs[(h, bi)][:, 2],
                                start=False, stop=(bi == 1))
                        rcp = pool.tile([64, 2 * T], f32, tag="rcp",
                                        name="rcp", bufs=3)
                        nc.vector.reciprocal_approx_fast(
                            rcp[:], av[0:64].rearrange("p a b -> p (a b)"))
                        nc.vector.tensor_tensor(
                            attnT[po:po + 64, hp, :],
                            av[64:128].rearrange("p a b -> p (a b)"),
                            rcp[:], ALU.mult)
                st["attnT"] = attnT

            def proj_ln2(st):
                attnT, xs = st["attnT"], st["xs"]
                hs = []
                for bi in range(2):
                    h_t = pool.tile([128, 2, C], bf16, tag="h", name="h_t",
                                    bufs=4)
                    for kt in range(2):
                        tk = bi * T + kt * 128
                        ps = ppool.tile([128, C], f32, tag="ps", name="pr_ps")
                        for ch in range(3):
                            nc.tensor.matmul(ps[:], attnT[:, ch, tk:tk + 128],
                                             wp[:, ch, :], start=(ch == 0),
                                             stop=(ch == 2 and not has_bp))
                        if has_bp:
                            nc.tensor.matmul(ps[:], ones8[0:1, 0:128], bpj,
                                             start=False, stop=True)
                        nc.vector.tensor_tensor(h_t[:, kt], ps[:],
                                                xs[bi][:, kt], ALU.add)
                    hs.append(h_t)
                srcs = [hs[bi][:, kt] for bi in range(2) for kt in range(2)]
                st["hs"] = hs
                st["ln2a"] = ln_stats_a(srcs, "ln2")

            def hnT_build(st, evac_dve=True):
                rstd4, nb4 = st.pop("ln2a")
                hnT = pool.tile([128, 3, 2 * T], bf16, tag="hnT", name="hnT")
                srcs = [st["hs"][bi][:, kt] for bi in range(2)
                        for kt in range(2)]
                lnT(hnT, srcs, rstd4, nb4, "ln2", evac_dve=evac_dve)
                st["hnT"] = hnT

            def ffn2_bi(st, bi):
                pair, hs = st["pair"], st["hs"]
                h1a, h1b = st["h1a"], st["h1b"]
                for b in (pair[bi],):
                    out_t = pool.tile([128, 2, C], f32, tag="out", name="out_t")
                    for kt in range(2):
                        tk = (2 * bi + kt) * 128
                        ps = ppool.tile([128, C], f32, tag="ps", name="f2_ps")
                        for j in range(6):
                            nc.tensor.matmul(
                                ps[:], h1a[:, j, tk:tk + 128],
                                w2a[:, j, :], start=(j == 0), stop=False)
                        for j in range(0, 6, 2):
                            nc.tensor.matmul(
                                ps[:], h1b[:, j:j + 2, tk:tk + 128],
                                w2b[:, j:j + 2, :], start=False,
                                stop=(j == 4 and not has_b2),
                                perf_mode=mybir.MatmulPerfMode.DoubleRow)
                        if has_b2:  # b2 pre-scaled x1024 on host
                            nc.tensor.matmul(ps[:], ones8[0:1, 0:128], b2b,
                                             start=False, stop=True)
                        nc.vector.scalar_tensor_tensor(
                            out_t[:, kt], ps[:], 1.0 / 1024.0, hs[bi][:, kt],
                            ALU.mult, ALU.add)
                        nc.sync.dma_start(out_d[b, kt * 128:(kt + 1) * 128, :],
                                          out_t[:, kt])

            def ffn2(st):
                ffn2_bi(st, 0)
                ffn2_bi(st, 1)
                st.pop("h1a")
                st.pop("h1b")
                st.pop("hnT")

            NP = BPC // 2
            sts = [ln1_dma(0), ln1_dma(1)]
            load_weights()
            st = tp_qkv(sts[0])
            prev = None
            for bp in range(NP):
                attention(st, prev)
                proj_ln2(st)
                nxt = tp_qkv(sts[bp + 1]) if bp + 1 < NP else None
                if prev is not None:
                    ffn2(prev)  # PE filler while the ln2 stats chain runs
                hnT_build(st, evac_dve=(bp + 1 < NP))
                if bp + 2 < NP:
                    sts.append(ln1_dma(bp + 2))
                prev = st
                st = nxt
            # tail: token-split ffn1 so ffn2(bi=0) overlaps ffn1's second half
            hnT, hs = prev["hnT"], prev["hs"]
            prev["h1a"] = pool.tile([128, 6, 2 * T], bf16, tag="h1a",
                                    name="h1a", bufs=1)
            prev["h1b"] = pool.tile([128, 6, 2 * T], f8, tag="h1b",
                                    name="h1b", bufs=1)
            for half in range(2):
                t0, t1 = half * T, half * T + T
                for mf in range(12):
                    ps = ppool.tile([128, T], f32, tag="ps", name="f1t_ps")
                    for c in range(3):
                        nc.tensor.matmul(ps[:], w1[:, c, mf * 128:(mf + 1) * 128],
                                         hnT[:, c, t0:t1], start=(c == 0),
                                         stop=(c == 2))
                    dst = prev["h1a"][:, mf, t0:t1] if mf < 6 else \
                        prev["h1b"][:, mf - 6, t0:t1]
                    sc8 = 1.0 if mf < 6 else 32.0
                    if mf % 2 == 0 or has_b1:
                        nc.scalar.activation(dst, ps[:], AF.Relu,
                                             bias=b1v[:, mf:mf + 1], scale=sc8)
                    else:  # only valid when b1 == 0
                        nc.vector.tensor_scalar(dst, ps[:], sc8, 0.0,
                                                ALU.mult, ALU.max)
                ffn2_bi(prev, half)

    nc.compile()
    return nc


def kernel(x, Wq, Wk, Wv, Wproj, bproj, W1, b1, W2, b2, ln1_g, ln1_b, ln2_g, ln2_b):
    import ml_dtypes

    x = np.asarray(x, dtype=np.float32)
    Wq = np.asarray(Wq, dtype=np.float32)
    Wk = np.asarray(Wk, dtype=np.float32)
    Wv = np.asarray(Wv, dtype=np.float32)
    Wproj = np.asarray(Wproj, dtype=np.float32)
    bproj = np.asarray(bproj, dtype=np.float32)
    W1 = np.asarray(W1, dtype=np.float32)
    b1 = np.asarray(b1, dtype=np.float32)
    W2 = np.asarray(W2, dtype=np.float32)
    b2 = np.asarray(b2, dtype=np.float32)
    ln1_g = np.asarray(ln1_g, dtype=np.float32)
    ln1_b = np.asarray(ln1_b, dtype=np.float32)
    ln2_g = np.asarray(ln2_g, dtype=np.float32)
    ln2_b = np.asarray(ln2_b, dtype=np.float32)

    # Fold LN gains into consuming weights; LN biases fold through weights.
    wq_h = np.ascontiguousarray(Wq.transpose(1, 0, 2).reshape(C, C) * ln1_g[:, None])
    wk_h = np.ascontiguousarray(Wk.transpose(1, 0, 2).reshape(C, C) * ln1_g[:, None])
    wv_h = np.ascontiguousarray(Wv.transpose(1, 0, 2).reshape(C, C) * ln1_g[:, None])
    qb_h = ln1_b @ wq_h
    kb_h = ln1_b @ wk_h
    vb_h = ln1_b @ wv_h
    w1_h = W1 * ln2_g[:, None]
    b1_h = np.ascontiguousarray(b1 + ln2_b @ w1_h)

    qb16 = lambda a: np.ascontiguousarray(a).astype(ml_dtypes.bfloat16)
    # [c_in, c_out] -> [p, c_plane, c_out] with c_in = c_plane*128 + p
    pcl = lambda w: np.ascontiguousarray(w.reshape(-1, 128, w.shape[-1]).transpose(1, 0, 2))
    wq8 = qb16(pcl(wq_h))
    wk8 = qb16(pcl(wk_h))
    wv8 = qb16(pcl(wv_h))
    wp8 = qb16(pcl(Wproj))
    w18 = qb16(pcl(w1_h))
    w2p = pcl(W2)
    w2a = qb16(w2p[:, 0:6, :] * 1024.0)
    w2b = _q8(w2p[:, 6:12, :] * 32.0)
    biases8 = qb16(np.stack([qb_h, kb_h, vb_h, bproj, b2 * 1024.0]))

    flags = tuple(bool(np.any(v)) for v in (qb_h, kb_h, vb_h, bproj, b2, b1_h))
    if flags not in _CACHE:
        _CACHE[flags] = _build(flags)
    nc = _CACHE[flags]

    shared = {"wq": wq8, "wk": wk8, "wv": wv8, "wp": wp8,
              "w1": w18, "w2a": w2a, "w2b": w2b, "b1": b1_h,
              "biases": biases8}
    in_maps = [{"x": np.ascontiguousarray(x[c * BPC:(c + 1) * BPC]),
                **shared}
               for c in range(N_CORES)]

    res = run_bass_kernel_spmd(nc, in_maps, list(range(N_CORES)), trace=TRACE)
    if TRACE:
        kernel.last_results = res
    return np.concatenate([res.results[c]["out"] for c in range(N_CORES)], axis=0)
